# revision 1
# baseline (speedup 1.0000x reference)
"""Trainium2 Bass kernel for CrossAttentionClusteringLearnableK.

Per-batch cross-attention block, data-parallel over B=8 across 8 NeuronCores.

  slots_norm = LN(slots)                       [64, 256]
  q = slots_norm @ Wq                          [64, 512]  (8 heads x 64)
  k = hits @ Wk ; v = hits @ Wv                [N, 512]
  att = softmax(q k^T / 8) v                   [64, 512]
  out = LN(slots + MLP(att))                   [64, 256]

Layout strategy (per core, transpose-free attention):
  - hitsT [256, N] built by PE-transposing hits tiles (bf16).
  - kT [512, N] = Wk.T @ hitsT  (head-major partitions, 2 heads per 128-tile)
  - logitsT [n, 512] = (kT-pair-tile).T @ blockdiag(qT_2h) -- one full-128-
    contraction matmul computes a 2-head pair; 4 matmuls cover 8 heads.
  - exp on ScalarE (no max subtraction: |logits| < ~1 by construction).
  - att_vT: contraction over n with lhsT = expT chunk, rhs = [v_h|v_h'|ones];
    the ones column accumulates the softmax denominators for free.
"""

import numpy as np

import concourse.bass as bass
import concourse.tile as tile
from concourse import bass_utils, mybir
from concourse.masks import make_identity

F32 = mybir.dt.float32
BF16 = mybir.dt.bfloat16

B, K, N = 8, 64, 16384
H, DH, DV, DQ, DHIT, DMLP = 8, 64, 64, 256, 256, 512
EPS = 1e-5
SC = 2048                 # n columns per super-chunk
NSC = N // SC             # 8
NT = SC // 128            # 16 chunk tiles per super-chunk
N_CORES = 8

INPUT_NAMES = [
    "slot_representations", "hit_features", "ln1_g", "ln1_b",
    "Wq", "Wk", "Wv", "W1", "b1", "W2", "b2", "ln2_g", "ln2_b",
]


def _split_waits(nc, max_waits=1):
    """walrus in this toolchain rejects >1 sync-wait on ctrl-less opcodes
    (Drain/NoOp); move excess waits onto preceding NoOps."""
    n_fix = 0
    for f in nc.m.functions:
        for bb in f.blocks:
            newlist = []
            changed = False
            for ins in bb.instructions:
                si = ins.sync_info
                if si is not None and si.on_wait and len(si.on_wait) > max_waits:
                    waits = list(si.on_wait)
                    extra, keep = waits[:-max_waits], waits[-max_waits:]
                    for i in range(0, len(extra), max_waits):
                        nop = mybir.InstNoOp(name=f"I-waitfix-{n_fix}", ins=[], outs=[])
                        n_fix += 1
                        nop.engine = ins.engine
                        nop.sync_info = mybir.SyncInfo(
                            on_wait=extra[i:i + max_waits], on_update=[])
                        newlist.append(nop)
                        nc.register_instruction(nop)
                    ins.sync_info = mybir.SyncInfo(
                        on_wait=keep, on_update=list(si.on_update))
                    changed = True
                newlist.append(ins)
            if changed:
                bb.instructions = newlist
    return n_fix


def _layernorm(nc, pool, out, x, g_b, b_b, p):
    """out = LN(x) * g + b over free dim; x [p, DQ] fp32 sbuf; g_b/b_b [p, DQ]."""
    stats = pool.tile([p, 6], F32, tag="ln_stats", bufs=2, name="ln_stats")
    nc.vector.bn_stats(out=stats, in_=x)
    mv = pool.tile([p, 2], F32, tag="ln_mv", bufs=2, name="ln_mv")
    nc.vector.bn_aggr(out=mv, in_=stats)
    std = pool.tile([p, 1], F32, tag="ln_std", bufs=2, name="ln_std")
    eps_t = pool.tile([p, 1], F32, tag="ln_eps", bufs=2, name="ln_eps")
    nc.vector.memset(eps_t, EPS)
    nc.scalar.activation(out=std, in_=mv[:, 1:2],
                         func=mybir.ActivationFunctionType.Sqrt,
                         bias=eps_t, scale=1.0)
    nc.vector.reciprocal(out=std, in_=std)
    nc.vector.tensor_scalar(out=out, in0=x, scalar1=mv[:, 0:1], scalar2=std,
                            op0=mybir.AluOpType.subtract,
                            op1=mybir.AluOpType.mult)
    nc.vector.tensor_mul(out=out, in0=out, in1=g_b)
    nc.vector.tensor_add(out=out, in0=out, in1=b_b)


def _bcast_row(t, p):
    """Broadcast a 1-D DRAM AP across p partitions."""
    return bass.AP(tensor=t.tensor if hasattr(t, "tensor") else t,
                   offset=t.offset if hasattr(t, "offset") else 0,
                   ap=[[0, p]] + list(t.ap if hasattr(t, "ap") else [[1, t.shape[0]]]))


def _body(nc, tc, pools, dram):
    const, wpool, work, psA, psB, psP, psT, psQ = (
        pools[k] for k in ("const", "w", "work", "psA", "psB", "psP", "psT", "psQ"))

    # ---------------- weights / constants prep ----------------
    ident = const.tile([128, 128], BF16, tag="ident", bufs=1, name="ident")
    make_identity(nc, ident)

    def load_cast(dram_t, rows, cols, tag):
        """Load fp32 [rows, cols] weight as rows//128 bf16 tiles [128, cols]."""
        tiles = []
        for c in range(rows // 128):
            stage = wpool.tile([128, cols], F32, tag="wstage", bufs=2, name="wstage")
            nc.sync.dma_start(out=stage, in_=dram_t[c * 128:(c + 1) * 128, :])
            t16 = wpool.tile([128, cols], BF16, tag=f"{tag}{c}", bufs=1,
                             name=f"{tag}{c}")
            nc.vector.tensor_copy(out=t16, in_=stage)
            tiles.append(t16)
        return tiles

    wq16 = load_cast(dram["Wq"], DQ, H * DH, "wq")
    wk16 = load_cast(dram["Wk"], DHIT, H * DH, "wk")
    wv16 = load_cast(dram["Wv"], DHIT, H * DV, "wv")
    w116 = load_cast(dram["W1"], H * DV, DMLP, "w1")
    w216 = load_cast(dram["W2"], DMLP, DQ, "w2")

    b1_sb = []
    for m in range(DMLP // 128):
        b1m = wpool.tile([128, 1], F32, tag=f"b1_{m}", bufs=1, name=f"b1_{m}")
        nc.gpsimd.dma_start(out=b1m, in_=dram["b1"][m * 128:(m + 1) * 128])
        b1_sb.append(b1m)
    b2_st = wpool.tile([1, DQ], F32, tag="b2_st", bufs=1, name="b2_st")
    nc.gpsimd.dma_start(out=b2_st, in_=dram["b2"][None, :])
    b2_16 = wpool.tile([1, DQ], BF16, tag="b2_16", bufs=1, name="b2_16")
    nc.vector.tensor_copy(out=b2_16, in_=b2_st)
    ones_row = const.tile([1, K], BF16, tag="ones_row", bufs=1, name="ones_row")
    nc.vector.memset(ones_row, 1.0)

    g1b = wpool.tile([K, DQ], F32, tag="g1b", bufs=1, name="g1b")
    nc.gpsimd.dma_start(out=g1b, in_=_bcast_row(dram["ln1_g"][:], K))
    b1b = wpool.tile([K, DQ], F32, tag="b1b", bufs=1, name="b1b")
    nc.gpsimd.dma_start(out=b1b, in_=_bcast_row(dram["ln1_b"][:], K))
    g2b = wpool.tile([K, DQ], F32, tag="g2b", bufs=1, name="g2b")
    nc.gpsimd.dma_start(out=g2b, in_=_bcast_row(dram["ln2_g"][:], K))
    b2b = wpool.tile([K, DQ], F32, tag="b2b", bufs=1, name="b2b")
    nc.gpsimd.dma_start(out=b2b, in_=_bcast_row(dram["ln2_b"][:], K))

    # ---------------- preamble: LN1, qT (block-diag, pre-scaled) ----------
    slots32 = work.tile([K, DQ], F32, tag="slots32", bufs=1, name="slots32")
    nc.sync.dma_start(out=slots32, in_=dram["slots"][:, :])
    sn32 = work.tile([K, DQ], F32, tag="sn32", bufs=1, name="sn32")
    _layernorm(nc, work, sn32, slots32, g1b, b1b, K)
    sn16 = work.tile([K, DQ], BF16, tag="sn16", bufs=1, name="sn16")
    nc.vector.tensor_copy(out=sn16, in_=sn32)

    snT = []
    for c in range(DQ // 128):
        tr = psT.tile([128, 256], BF16, tag="ps_tr", bufs=2, name="ps_tr")
        nc.tensor.transpose(tr[:, 0:K], sn16[:, c * 128:(c + 1) * 128],
                            ident[0:64, 0:64])
        s16 = work.tile([128, K], BF16, tag=f"snT{c}", bufs=1, name=f"snT{c}")
        nc.vector.tensor_copy(out=s16, in_=tr[:, 0:K])
        snT.append(s16)

    qTd = []   # block-diag qT per head pair, scaled by DH^-0.5
    for m in range(4):
        q_ps = psP.tile([128, K], F32, tag="ps_proj", bufs=2, name="ps_proj")
        nc.tensor.matmul(q_ps, wq16[0][:, m * 128:(m + 1) * 128], snT[0],
                         start=True, stop=False)
        nc.tensor.matmul(q_ps, wq16[1][:, m * 128:(m + 1) * 128], snT[1],
                         start=False, stop=True)
        qd = work.tile([128, 128], BF16, tag=f"qTd{m}", bufs=1, name=f"qTd{m}")
        nc.vector.memset(qd, 0.0)
        nc.scalar.activation(out=qd[0:64, 0:64], in_=q_ps[0:64, :],
                             func=mybir.ActivationFunctionType.Copy,
                             scale=DH ** -0.5)
        nc.scalar.activation(out=qd[64:128, 64:128], in_=q_ps[64:128, :],
                             func=mybir.ActivationFunctionType.Copy,
                             scale=DH ** -0.5)
        qTd.append(qd)

    # Fuse the k-projection into QK: wqk[m] = Wk[:, pair m] @ qTd[m]  [256,128]
    # so logitsT chunk = hitsT_chunk.T @ wqk[m] with full-256 contraction.
    wqk16 = []
    for m in range(4):
        percol = []
        for c in range(2):
            wkT_ps = psT.tile([128, 256], BF16, tag="ps_tr", bufs=2, name="ps_tr")
            nc.tensor.transpose(wkT_ps[:, 0:128],
                                wk16[c][:, m * 128:(m + 1) * 128], ident)
            wkT = work.tile([128, 128], BF16, tag="wkT", bufs=2, name="wkT")
            nc.vector.tensor_copy(out=wkT, in_=wkT_ps[:, 0:128])
            wq_ps = psP.tile([128, 512], F32, tag="ps_proj", bufs=2,
                             name="ps_proj")
            nc.tensor.matmul(wq_ps[:, 0:128], wkT, qTd[m], start=True, stop=True)
            w16 = work.tile([128, 128], BF16, tag=f"wqk{m}_{c}", bufs=1,
                            name=f"wqk{m}_{c}")
            nc.vector.tensor_copy(out=w16, in_=wq_ps[:, 0:128])
            percol.append(w16)
        wqk16.append(percol)

    # attv accumulator [128, 4 pairs, 129] fp32 (col 128 = softmax denom)
    attv_acc = work.tile([128, 4, 129], F32, tag="attv_acc", bufs=1,
                         name="attv_acc")

    # ---------------- streaming attention over n ----------------
    for s in range(NSC):
        n0 = s * SC
        hitsT = work.tile([128, 2, SC], BF16, tag="hitsT", bufs=2, name="hitsT")
        for tt in range(NT // 2):
            h32 = work.tile([128, 2, DHIT], F32, tag="h32", bufs=3, name="h32")
            nc.sync.dma_start(
                out=h32,
                in_=dram["hits"][n0 + tt * 256:n0 + (tt + 1) * 256, :]
                .rearrange("(a p) d -> p a d", p=128))
            h16 = work.tile([128, 2, DHIT], BF16, tag="h16", bufs=3, name="h16")
            nc.gpsimd.tensor_copy(out=h16, in_=h32)
            for a in range(2):
                t = tt * 2 + a
                trp = psT.tile([128, 256], BF16, tag="ps_tr", bufs=2, name="ps_tr")
                nc.tensor.transpose(trp[:, 0:128], h16[:, a, 0:128], ident)
                nc.tensor.transpose(trp[:, 128:256], h16[:, a, 128:256], ident)
                nc.vector.tensor_copy(
                    out=hitsT[:, :, t * 128:(t + 1) * 128],
                    in_=trp[:].rearrange("p (c x) -> p c x", c=2))

        # v [SC, 512] -> [128, t, 4 pairs, 130] with ones col at 128
        v16 = work.tile([128, NT, 4, 130], BF16, tag="v16", bufs=2, name="v16")
        nc.vector.memset(v16[:, :, :, 128:129], 1.0)
        for t in range(NT):
            vps = psP.tile([128, 512], F32, tag="ps_proj", bufs=2, name="ps_proj")
            nc.tensor.matmul(vps, hitsT[:, 0, t * 128:(t + 1) * 128], wv16[0],
                             start=True, stop=False)
            nc.tensor.matmul(vps, hitsT[:, 1, t * 128:(t + 1) * 128], wv16[1],
                             start=False, stop=True)
            ev_engine = nc.vector if t % 2 == 0 else nc.scalar
            if t % 2 == 0:
                nc.vector.tensor_copy(
                    out=v16[:, t, :, 0:128],
                    in_=vps[:].rearrange("p (c x) -> p c x", c=4))
            else:
                nc.scalar.copy(
                    out=v16[:, t, :, 0:128],
                    in_=vps[:].rearrange("p (c x) -> p c x", c=4))

        # QK^T + exp + att_v per 128-chunk
        attv_A = psA.tile([128, 2, 129], F32, tag="attv_A", bufs=1, name="attv_A")
        attv_B = psB.tile([128, 2, 129], F32, tag="attv_B", bufs=1, name="attv_B")
        # software-pipelined: att_v for chunk t-1 is emitted after QK(t) so
        # PE never stalls on the ScalarE exp of the current chunk.
        exq = []
        def emit_attv(tp, ext):
            for m in range(4):
                dst = attv_A if m < 2 else attv_B
                nc.tensor.matmul(dst[:, m % 2, :],
                                 ext[:, m * 128:(m + 1) * 128],
                                 v16[:, tp, m, 0:129],
                                 start=(tp == 0), stop=(tp == NT - 1),
                                 skip_group_check=True)
        for t in range(NT):
            qk = psQ.tile([128, 512], F32, tag="qk", bufs=2, name="qk")
            for m in range(4):
                nc.tensor.matmul(qk[:, m * 128:(m + 1) * 128],
                                 hitsT[:, 0, t * 128:(t + 1) * 128],
                                 wqk16[m][0], start=True, stop=False)
                nc.tensor.matmul(qk[:, m * 128:(m + 1) * 128],
                                 hitsT[:, 1, t * 128:(t + 1) * 128],
                                 wqk16[m][1], start=False, stop=True)
            ex = work.tile([128, 512], BF16, tag="ex", bufs=3, name="ex")
            nc.scalar.activation(out=ex, in_=qk,
                                 func=mybir.ActivationFunctionType.Exp)
            exq.append((t, ex))
            if len(exq) > 1:
                emit_attv(*exq.pop(0))
        emit_attv(*exq.pop(0))
        if s == 0:
            nc.vector.tensor_copy(out=attv_acc[:, 0:2, :], in_=attv_A)
            nc.vector.tensor_copy(out=attv_acc[:, 2:4, :], in_=attv_B)
        else:
            nc.vector.tensor_add(out=attv_acc[:, 0:2, :],
                                 in0=attv_acc[:, 0:2, :], in1=attv_A)
            nc.vector.tensor_add(out=attv_acc[:, 2:4, :],
                                 in0=attv_acc[:, 2:4, :], in1=attv_B)

    # ---------------- normalize + aoT + MLP + residual LN2 ----------------
    aoT = []
    for m in range(4):
        rec = work.tile([128, 1], F32, tag="rec", bufs=2, name="rec")
        nc.vector.reciprocal(out=rec, in_=attv_acc[:, m, 128:129])
        avn = work.tile([128, 128], BF16, tag="avn", bufs=2, name="avn")
        nc.vector.tensor_scalar_mul(out=avn, in0=attv_acc[:, m, 0:128],
                                    scalar1=rec)
        trp = psT.tile([128, 256], BF16, tag="ps_tr", bufs=2, name="ps_tr")
        trp = trp[:, 0:128]
        nc.tensor.transpose(trp, avn, ident)
        at = work.tile([128, K], BF16, tag=f"aoT{m}", bufs=1, name=f"aoT{m}")
        nc.scalar.copy(out=at[0:64, :], in_=trp[0:64, 0:64])
        nc.scalar.copy(out=at[64:128, :], in_=trp[64:128, 64:128])
        aoT.append(at)

    h1T = []
    for m in range(DMLP // 128):
        hps = psP.tile([128, K], F32, tag="ps_proj", bufs=2, name="ps_proj")
        for c in range(4):
            nc.tensor.matmul(hps, w116[c][:, m * 128:(m + 1) * 128], aoT[c],
                             start=(c == 0), stop=(c == 3))
        ht = work.tile([128, K], BF16, tag=f"h1T{m}", bufs=1, name=f"h1T{m}")
        nc.scalar.activation(out=ht, in_=hps,
                             func=mybir.ActivationFunctionType.Relu,
                             bias=b1_sb[m])
        h1T.append(ht)

    mlp_ps = psP.tile([K, DQ], F32, tag="ps_proj", bufs=2, name="ps_proj")
    for m in range(DMLP // 128):
        nc.tensor.matmul(mlp_ps, h1T[m], w216[m], start=(m == 0), stop=False)
    nc.tensor.matmul(mlp_ps, ones_row, b2_16, start=False, stop=True)

    res32 = work.tile([K, DQ], F32, tag="res32", bufs=1, name="res32")
    nc.vector.tensor_add(out=res32, in0=slots32, in1=mlp_ps)
    out32 = work.tile([K, DQ], F32, tag="out32", bufs=2, name="out32")
    _layernorm(nc, work, out32, res32, g2b, b2b, K)
    nc.sync.dma_start(out=dram["out"][:, :], in_=out32)


def build_nc(n_reps=1):
    nc = bass.Bass()
    dram = {}
    dram["slots"] = nc.declare_dram_parameter(
        "slots", [K, DQ], F32, isOutput=False)
    dram["hits"] = nc.declare_dram_parameter(
        "hits", [N, DHIT], F32, isOutput=False)
    for nm, shape in [("ln1_g", [DQ]), ("ln1_b", [DQ]),
                      ("Wq", [DQ, H * DH]), ("Wk", [DHIT, H * DH]),
                      ("Wv", [DHIT, H * DV]), ("W1", [H * DV, DMLP]),
                      ("b1", [DMLP]), ("W2", [DMLP, DQ]), ("b2", [DQ]),
                      ("ln2_g", [DQ]), ("ln2_b", [DQ])]:
        dram[nm] = nc.declare_dram_parameter(nm, shape, F32, isOutput=False)
    dram["out"] = nc.declare_dram_parameter("out", [K, DQ], F32, isOutput=True)

    with tile.TileContext(nc) as tc:
        import contextlib
        with contextlib.ExitStack() as ctx:
            pools = {
                "const": ctx.enter_context(tc.tile_pool(name="const", bufs=1)),
                "w": ctx.enter_context(tc.tile_pool(name="w", bufs=1)),
                "work": ctx.enter_context(tc.tile_pool(name="work", bufs=1)),
                "psA": ctx.enter_context(
                    tc.tile_pool(name="psA", bufs=1, space="PSUM")),
                "psB": ctx.enter_context(
                    tc.tile_pool(name="psB", bufs=1, space="PSUM")),
                "psP": ctx.enter_context(
                    tc.tile_pool(name="psP", bufs=2, space="PSUM")),
                "psT": ctx.enter_context(
                    tc.tile_pool(name="psT", bufs=2, space="PSUM")),
                "psQ": ctx.enter_context(
                    tc.tile_pool(name="psQ", bufs=2, space="PSUM")),
            }
            for _ in range(n_reps):
                _body(nc, tc, pools, dram)
    _split_waits(nc)
    return nc


_NC_CACHE = {}


def _input_map(inputs, core):
    m = {"slots": np.ascontiguousarray(inputs["slot_representations"][core]),
         "hits": np.ascontiguousarray(inputs["hit_features"][core])}
    for nm in INPUT_NAMES[2:]:
        m[nm] = np.ascontiguousarray(np.asarray(inputs[nm], dtype=np.float32))
    return m


def run(inputs, n_reps=1):
    if n_reps not in _NC_CACHE:
        _NC_CACHE[n_reps] = build_nc(n_reps)
    nc = _NC_CACHE[n_reps]
    core_ids = list(range(N_CORES))
    in_maps = [_input_map(inputs, i) for i in core_ids]
    res = bass_utils.run_bass_kernel_spmd(nc, in_maps, core_ids)
    out = np.stack([res.results[i]["out"] for i in core_ids]).astype(np.float32)
    return out


def kernel(**inputs):
    return run(inputs, n_reps=1)

